# revision 28
# baseline (speedup 1.0000x reference)
"""Bidirectional attention kernel for Trainium2 (8 NeuronCores, data-parallel over batch).

Math per example (B=32, L1=L2=512, D=1024):
    sim = v1 @ v2^T                                  [512, 512]
    attn1 = softmax_j(sim + v2maskbias)              (mask v2 cols)
    attn2 = softmax_i(sim + v1maskbias)              (mask v1 rows)
    out1  = (attn1 @ v2) zeroed at v1-masked rows    [512, 1024]
    out2  = (attn2^T @ v1) zeroed at v2-masked rows  [512, 1024]

Device strategy (4 examples per core), all-16-bit datapath:
  - sim matmul operands in fp16 (host pre-transposed); PSUM fp32. fp16
    logits keep softmax ties stable (bf16 does not: 9e-2 rel err).
  - e1 numerators fp16 (range (0,1]); e2 numerators bf16 (range up to
    e^60 from the global-max-bound trick, needs fp32 exponent range).
  - attend rhs: v2 natural fp16 (out1), v1 natural bf16 (out2). Outputs
    bf16, unpacked/upcast on host.
  - Host packs each per-example tensor into a single [128, x] row-major
    region so loads use few, large-descriptor DMAs (DMA issues cost
    ~700ns each on the issuing engine and serialize; the DMA subsystem
    also ramps over the first ~10us of wall clock). All bulk loads ride
    sync's HW DGE queue in strict need order: example 0's sim operands
    in fine chunks (compute tracks the ramp-limited arrivals), example
    1 in quarters, later examples in halves, each example's natural-
    layout attend operands AFTER the next example's sim operands (they
    are consumed one stage later). Scalar's queue carries only small
    early items (bias rows, consts). Example 0's sim runs as two half-K
    passes (partial sums parked in SBUF, bias folded into the first
    add) so all four row blocks compute as soon as chunks 0-3 land.
  - Tail: s2 ones-matmuls + stats hoisted before att1 so the reciprocal
    chain hides under att1; last example's o2 halves store via the idle
    sync/scalar HW queues (gpsimd's software-DGE backlog would gate the
    drain), and the final block's copy splits across DVE+ACT with two
    parallel quarter-stores.
  - e1 -> e1ji transpose on the PE (fp16 transposes run 1 cyc/row, half
    the fp32 cost); PSUM->SBUF copies balanced across ACT and DVE
    (gpsimd/Pool cannot touch PSUM); store issues on gpsimd.
  - Row softmax stats ride the EXP activation accumulator; 1/sum and
    mask-zeroing fold into PSUM->SBUF output copies (per-partition
    scale) which are spread across ACT/DVE/Pool to keep all three off
    the critical path. Column sums for attn2 via tiny ones-matmuls.
  - 1-example software-pipeline skew: sim+softmax of example e issue
    before the attend matmuls of example e-1 so the PE never waits on
    the softmax stats chain and stays at full p-state.
"""

import numpy as np

B, L, D = 32, 512, 1024
NCORES = 8
EPC = B // NCORES  # examples per core
NB = L // 128      # 128-row blocks per L
ND = D // 128      # 128-row chunks per D (transposed layouts)
NDC = D // 512     # 512-col halves per D

_CACHE = {}
LAST_RESULTS = None


def _build_nc():
    from contextlib import ExitStack
    import concourse.bacc as bacc
    import concourse.tile as tile
    import concourse.mybir as mybir
    import concourse.bass_isa as bass_isa

    f32 = mybir.dt.float32
    f16 = mybir.dt.float16
    bf16 = mybir.dt.bfloat16
    EXP = mybir.ActivationFunctionType.Exp
    COPY = mybir.ActivationFunctionType.Copy
    ADD = mybir.AluOpType.add
    MIN = mybir.AluOpType.min
    MAX = mybir.AluOpType.max
    AXX = mybir.AxisListType.X

    nc = bacc.Bacc("TRN2", target_bir_lowering=False, debug=False, num_devices=NCORES)
    # packed layouts: one [128, x] row-major region per example per tensor
    v1td = nc.dram_tensor("v1t", [EPC * 128, ND * L], f16, kind="ExternalInput")
    v2td = nc.dram_tensor("v2t", [EPC * 128, ND * L], f16, kind="ExternalInput")
    v2nd = nc.dram_tensor("v2n", [EPC * 128, NB * D], f16, kind="ExternalInput")
    v1nd = nc.dram_tensor("v1n", [EPC * 128, NB * D], bf16, kind="ExternalInput")
    b2d = nc.dram_tensor("b2r", [EPC * 128, L], f32, kind="ExternalInput")
    cmd = nc.dram_tensor("cm", [128, 2 * EPC * NB], f32, kind="ExternalInput")
    bcd = nc.dram_tensor("bcol", [128, EPC * NB], f32, kind="ExternalInput")
    idd = nc.dram_tensor("idh", [128, 128], f16, kind="ExternalInput")
    ond = nc.dram_tensor("ones2", [128, 2], bf16, kind="ExternalInput")
    o1d = nc.dram_tensor("o1", [EPC * 128, NB * D], bf16, kind="ExternalOutput")
    o2d = nc.dram_tensor("o2", [EPC * 128, NB * D], bf16, kind="ExternalOutput")
    v1ta, v2ta, v2na, v1na = v1td.ap(), v2td.ap(), v2nd.ap(), v1nd.ap()
    o1a, o2a = o1d.ap(), o2d.ap()

    with ExitStack() as ctx:
        # (Tried: pre-TileContext warm-up DMAs to start the DMA ramp early —
        # the ramp turned out to be wall-clock-tied, not first-work-tied, so
        # the warmup only displaced the real loads. Reverted.)
        tc = ctx.enter_context(tile.TileContext(nc))
        const = ctx.enter_context(tc.tile_pool(name="const", bufs=1))
        pv = ctx.enter_context(tc.tile_pool(name="pv", bufs=1))
        pvt = ctx.enter_context(tc.tile_pool(name="pvt", bufs=1))
        pe_ = ctx.enter_context(tc.tile_pool(name="pe", bufs=1))
        pst = ctx.enter_context(tc.tile_pool(name="pst", bufs=1))
        pbb = ctx.enter_context(tc.tile_pool(name="pbb", bufs=1))
        pav = ctx.enter_context(tc.tile_pool(name="pav", bufs=1))
        pps = ctx.enter_context(tc.tile_pool(name="pps", bufs=1, space="PSUM"))

        st = [dict() for _ in range(EPC)]

        def load_warmup():
            # The scalar DMA queue's descriptor pipeline ramps up over ~3us;
            # give it a small const first so b2 (needed ~12us) rides a warm
            # queue. Sync's queue starts directly with the sim-gating chunk
            # (a warmup there just delays the first matmul).
            cms = const.tile([128, 2 * EPC * NB], f32)
            nc.scalar.dma_start(out=cms, in_=cmd.ap())
            return cms

        def load_consts():
            # needed ~15us in (first transpose / stats); after b2 on scalar.
            ident = const.tile([128, 128], f16)
            nc.scalar.dma_start(out=ident, in_=idd.ap())
            bcs = const.tile([128, EPC * NB], f32)
            nc.scalar.dma_start(out=bcs, in_=bcd.ap())
            onesr = const.tile([128, 2], bf16)
            nc.scalar.dma_start(out=onesr, in_=ond.ap())
            return ident, bcs, onesr

        def stage_load(e):
            # DMA-issue instructions cost ~700ns each and serialize per
            # queue; transfers also drain per queue roughly in order while
            # both queues share the 16 DMA engines. So: the sim-critical
            # stream (v1T/v2T interleaved in consumption order, chunked so
            # completion semaphores fire at chunk granularity) gets sync's
            # queue to itself at startup; example 0's small early items
            # (b2, consts) ride scalar's queue.
            s = st[e]
            v1T = pvt.tile([128, ND * L], f16, tag="v1T", bufs=2, name=f"v1T_{e}")
            v2T = pvt.tile([128, ND * L], f16, tag="v2T", bufs=2, name=f"v2T_{e}")
            b2bc0 = pbb.tile([128, L], f32, tag="b2", bufs=2, name=f"b2bc_{e}")
            if e == 0:
                cuts = [0, L, 2 * L, 4 * L, 6 * L, ND * L]
            elif e == 1:
                cuts = [0, 2 * L, 4 * L, 6 * L, ND * L]
            else:
                cuts = [0, 4 * L, ND * L]
            # All bulk loads ride sync's queue in strict need order. (Tried:
            # splitting v1T/v2T across sync+scalar — regresses ~8us because
            # scalar's DMA issues serialize with ACT's exp/copy work
            # mid-body, delaying both the loads and the softmax chain.)
            for a, b in zip(cuts[:-1], cuts[1:]):
                nc.sync.dma_start(out=v1T[:, a:b], in_=v1ta[e * 128 : (e + 1) * 128, a:b])
                nc.sync.dma_start(out=v2T[:, a:b], in_=v2ta[e * 128 : (e + 1) * 128, a:b])
            beng = nc.scalar if e == 0 else nc.sync
            beng.dma_start(out=b2bc0, in_=b2d.ap()[e * 128 : (e + 1) * 128, :])
            s["v1T"] = v1T
            s["v2T"] = v2T
            s["b2bc"] = b2bc0
            # natural-layout attend operands of the PREVIOUS example ride
            # after this example's sim-critical pieces: attend(e-1) runs
            # after sim(e) on the PE, so this ordering matches need order
            # and keeps the DMA-ramp window clear for the sim stream.
            if e > 0:
                stage_load_nat(e - 1)
            if e == EPC - 1:
                stage_load_nat(e)

        def stage_load_nat(e):
            s = st[e]
            v2n = pv.tile([128, NB * D], f16, tag="v2n", bufs=2, name=f"v2n_{e}")
            nc.sync.dma_start(out=v2n, in_=v2na[e * 128 : (e + 1) * 128, :])
            v1n = pv.tile([128, NB * D], bf16, tag="v1n", bufs=2, name=f"v1n_{e}")
            nc.sync.dma_start(out=v1n, in_=v1na[e * 128 : (e + 1) * 128, :])
            s["v2n"] = v2n
            s["v1n"] = v1n

        def stage_sim(e):
            s = st[e]
            m1nt = pst.tile([128, NB], f32, tag="m1nt", bufs=2, name=f"m1nt_{e}")
            s1t = pst.tile([128, NB], f32, tag="s1t", bufs=2, name=f"s1t_{e}")
            s["mk"], s["e1"], s["e2"] = [], [], []
            if e == 0:
                # example 0's operands stream through the cold DMA ramp.
                # Run FOUR concurrent psum chains (2 borrowed from the att
                # ring, idle until finB(0)) in chunk-arrival order: every
                # arriving chunk-pair immediately feeds all four row blocks,
                # and each chain's logits finish as soon as the last chunk
                # lands instead of serializing chain-by-chain.
                psl = [pps.tile([128, L], f32, tag="sim", bufs=2,
                                name=f"sim0_{i}") for i in range(2)]
                psl += [pps.tile([128, 512], f32, tag="att", bufs=3,
                                 name=f"sim0b_{i}") for i in range(2)]
                for c in range(ND):
                    for ib in range(NB):
                        nc.tensor.matmul(
                            psl[ib],
                            s["v1T"][:, c * L + ib * 128 : c * L + (ib + 1) * 128],
                            s["v2T"][:, c * L : (c + 1) * L],
                            start=(c == 0),
                            stop=(c == ND - 1),
                        )
            for ib in range(NB):
                if e == 0:
                    ps = psl[ib]
                else:
                    ps = pps.tile([128, L], f32, tag="sim", bufs=2)
                    for c in range(ND):
                        nc.tensor.matmul(
                            ps,
                            s["v1T"][:, c * L + ib * 128 : c * L + (ib + 1) * 128],
                            s["v2T"][:, c * L : (c + 1) * L],
                            start=(c == 0),
                            stop=(c == ND - 1),
                        )
                mk = pe_.tile([128, L], f32, tag="mk", bufs=2 * NB, name=f"mk_{e}_{ib}")
                nc.vector.tensor_add(mk, ps, s["b2bc"])
                # m1n = -rowmax(mk): the e1 exp bias, negated in one op
                nc.vector.tensor_reduce(m1nt[:, ib : ib + 1], mk, axis=AXX, op=MAX,
                                        negate=True)
                e1 = pe_.tile([128, L], f16, tag="e1", bufs=2 * NB, name=f"e1_{e}_{ib}")
                nc.scalar.activation(out=e1, in_=mk, func=EXP,
                                     bias=m1nt[:, ib : ib + 1], scale=1.0,
                                     accum_out=s1t[:, ib : ib + 1])
                s["mk"].append(mk)
                s["e1"].append(e1)
            # gm = global max = -min(m1n); all-reduce across partitions
            gmx = pst.tile([128, 1], f32, tag="gmx", bufs=2, name=f"gmx_{e}")
            nc.vector.tensor_reduce(gmx, m1nt, axis=AXX, op=MIN, negate=True)
            gmr = pst.tile([128, 1], f32, tag="gmr", bufs=2, name=f"gmr_{e}")
            nc.gpsimd.partition_all_reduce(gmr, gmx, 128, bass_isa.ReduceOp.max)
            # bias = 60 - gm keeps e2 numerators in normal fp32/bf16 range
            gmn = pst.tile([128, 1], f32, tag="gmn", bufs=2, name=f"gmn_{e}")
            nc.vector.tensor_scalar(gmn, gmr, -1.0, 60.0, op0=mybir.AluOpType.mult,
                                    op1=ADD)
            comb2 = pst.tile([128, NB], f32, tag="comb2", bufs=2, name=f"comb2_{e}")
            nc.vector.tensor_scalar_add(comb2, bcs[:, e * NB : e * NB + NB], gmn)
            r1t = pst.tile([128, NB], f32, tag="r1t", bufs=2, name=f"r1t_{e}")
            nc.vector.reciprocal(out=r1t, in_=s1t)
            sc1t = pst.tile([128, NB], f32, tag="sc1t", bufs=2, name=f"sc1t_{e}")
            nc.vector.tensor_mul(sc1t, r1t, cms[:, e * NB : e * NB + NB])
            s["sc1t"] = sc1t
            # e2 = exp(mk + b1col - gm + 60); b2row term cancels per-column
            for ib in range(NB):
                e2 = pe_.tile([128, L], bf16, tag="e2", bufs=2 * NB, name=f"e2_{e}_{ib}")
                nc.scalar.activation(out=e2, in_=s["mk"][ib], func=EXP,
                                     bias=comb2[:, ib : ib + 1], scale=1.0)
                s["e2"].append(e2)

        def stage_finA(e):
            # transpose e1 into [j,i] lhsT layout (fp16 PE transposes).
            # Emitted BEFORE sim(e+1) so the DVE copies don't queue behind
            # the next example's softmax work (e1 deps are long satisfied).
            s = st[e]
            e1ji = pe_.tile([128, NB * L], f16, tag="e1ji", bufs=2, name=f"e1ji_{e}")
            for jb in range(NB):
                ps = pps.tile([128, L], f16, tag="pte", bufs=2, name=f"pt1_{e}_{jb}")
                for ib in range(NB):
                    nc.tensor.transpose(
                        ps[:, ib * 128 : (ib + 1) * 128],
                        s["e1"][ib][:, jb * 128 : (jb + 1) * 128],
                        ident,
                    )
                # both engines copy half each so the pte PSUM bank frees in
                # half the time (transpose group jb+2 WAR-waits on this copy)
                nc.vector.tensor_copy(e1ji[:, jb * L : jb * L + 256], ps[:, 0:256])
                nc.scalar.copy(e1ji[:, jb * L + 256 : (jb + 1) * L], ps[:, 256:512])
            s["e1ji"] = e1ji

        def stage_finB(e):
            s = st[e]
            e1ji = s["e1ji"]
            # ---- s2 column sums via ones-matmuls on e2 tiles. Hoisted BEFORE
            # att1 so the s2 -> 1/s2 -> sc2t DVE chain hides under att1's
            # ~7us of matmuls instead of gating the att2 output copies.
            pss = pps.tile([128, 2 * NB], f32, tag="pss", bufs=1, name=f"pss_{e}")
            for jb in range(NB):
                for ib in range(NB):
                    nc.tensor.matmul(pss[:, 2 * jb : 2 * jb + 2],
                                     s["e2"][ib][:, jb * 128 : (jb + 1) * 128], onesr,
                                     start=(ib == 0), stop=(ib == NB - 1))
            s2t = pst.tile([128, NB], f32, tag="s2t", bufs=2, name=f"s2t_{e}")
            nc.vector.tensor_scalar_add(s2t, pss[:, 0 : 2 * NB : 2], 1.0e-36)
            r2t = pst.tile([128, NB], f32, tag="r2t", bufs=2, name=f"r2t_{e}")
            nc.vector.reciprocal(out=r2t, in_=s2t)
            sc2t = pst.tile([128, NB], f32, tag="sc2t", bufs=2, name=f"sc2t_{e}")
            nc.vector.tensor_mul(sc2t, r2t, cms[:, EPC * NB + e * NB : EPC * NB + e * NB + NB])
            # ---- out1[i,d] = sum_j e1[j,i] v2[j,d] / s1, masked rows zeroed
            for ib in range(NB):
                av = pav.tile([128, D], bf16, tag="av1", bufs=6)
                for dc in range(NDC):
                    ps = pps.tile([128, 512], f32, tag="att", bufs=3)
                    for jb in range(NB):
                        nc.tensor.matmul(
                            ps,
                            e1ji[:, jb * L + ib * 128 : jb * L + (ib + 1) * 128],
                            s["v2n"][:, jb * D + dc * 512 : jb * D + (dc + 1) * 512],
                            start=(jb == 0),
                            stop=(jb == NB - 1),
                        )
                    dst = av[:, dc * 512 : (dc + 1) * 512]
                    if dc == 0:
                        nc.scalar.activation(out=dst, in_=ps, func=COPY,
                                             scale=s["sc1t"][:, ib : ib + 1])
                    else:
                        nc.vector.tensor_scalar_mul(dst, ps, s["sc1t"][:, ib : ib + 1])
                nc.gpsimd.dma_start(
                    out=o1a[e * 128 : (e + 1) * 128, ib * D : (ib + 1) * D], in_=av)
            # ---- out2[j,d] = sum_i e2[i,j] v1[i,d] / s2, masked rows zeroed
            for jb in range(NB):
                av = pav.tile([128, D], bf16, tag="av2", bufs=6)
                for dc in range(NDC):
                    ps = pps.tile([128, 512], f32, tag="att", bufs=3)
                    for ib in range(NB):
                        nc.tensor.matmul(
                            ps,
                            s["e2"][ib][:, jb * 128 : (jb + 1) * 128],
                            s["v1n"][:, ib * D + dc * 512 : ib * D + (dc + 1) * 512],
                            start=(ib == 0),
                            stop=(ib == NB - 1),
                        )
                    dst = av[:, dc * 512 : (dc + 1) * 512]
                    last = e == EPC - 1 and jb == NB - 1 and dc == NDC - 1
                    if last:
                        # very last block of the kernel: split the copy
                        # across DVE and ACT so the final store chain starts
                        # ~400ns earlier.
                        nc.vector.tensor_scalar_mul(dst[:, 0:256], ps[:, 0:256],
                                                    sc2t[:, jb : jb + 1])
                        nc.scalar.activation(out=dst[:, 256:512], in_=ps[:, 256:512],
                                             func=COPY, scale=sc2t[:, jb : jb + 1])
                    elif dc == 0:
                        nc.vector.tensor_scalar_mul(dst, ps, sc2t[:, jb : jb + 1])
                    else:
                        nc.scalar.activation(out=dst, in_=ps, func=COPY,
                                             scale=sc2t[:, jb : jb + 1])
                    if e == EPC - 1:
                        # kernel-tail drain: store each half as soon as its
                        # copy lands, on the idle hardware DGE queues (sync
                        # for the DVE-copied half, scalar for the ACT-copied
                        # half) so issue+transfer parallelize instead of
                        # queueing behind gpsimd's software-DGE backlog.
                        base = e * 128
                        if last:
                            nc.sync.dma_start(
                                out=o2a[base : base + 128,
                                        jb * D + dc * 512 : jb * D + dc * 512 + 256],
                                in_=dst[:, 0:256])
                            nc.scalar.dma_start(
                                out=o2a[base : base + 128,
                                        jb * D + dc * 512 + 256 : jb * D + (dc + 1) * 512],
                                in_=dst[:, 256:512])
                        else:
                            seng = nc.sync if dc == 0 else nc.scalar
                            seng.dma_start(
                                out=o2a[base : base + 128,
                                        jb * D + dc * 512 : jb * D + (dc + 1) * 512],
                                in_=dst)
                if e != EPC - 1:
                    nc.gpsimd.dma_start(
                        out=o2a[e * 128 : (e + 1) * 128, jb * D : (jb + 1) * D], in_=av)
            st[e] = {}

        cms = load_warmup()
        stage_load(0)
        ident, bcs, onesr = load_consts()
        stage_sim(0)
        for e in range(1, EPC):
            stage_load(e)
            stage_finA(e - 1)
            stage_sim(e)
            stage_finB(e - 1)
        stage_finA(EPC - 1)
        stage_finB(EPC - 1)

    nc.compile()
    return nc


def get_nc():
    if "nc" not in _CACHE:
        _CACHE["nc"] = _build_nc()
    return _CACHE["nc"]


def _host_prep(v1, v2, v1_mask, v2_mask):
    """Build per-core input maps (packed per-example layouts) from full inputs."""
    import ml_dtypes

    bf16 = ml_dtypes.bfloat16
    v1 = np.asarray(v1, dtype=np.float32)
    v2 = np.asarray(v2, dtype=np.float32)
    v1_mask = np.asarray(v1_mask).astype(bool)
    v2_mask = np.asarray(v2_mask).astype(bool)

    def pack_t(x):  # [EPC, L, D] -> [EPC*128, ND*L]; row p = concat chunk rows
        return np.ascontiguousarray(
            x.transpose(0, 2, 1).reshape(EPC, ND, 128, L).transpose(0, 2, 1, 3)
            .reshape(EPC * 128, ND * L))

    def pack_n(x):  # [EPC, L, D] -> [EPC*128, NB*D]; row p = concat block rows
        return np.ascontiguousarray(
            x.reshape(EPC, NB, 128, D).transpose(0, 2, 1, 3).reshape(EPC * 128, NB * D))

    in_maps = []
    for k in range(NCORES):
        sl = slice(EPC * k, EPC * (k + 1))
        v1c, v2c = v1[sl], v2[sl]
        m1 = v1_mask[sl]
        m2 = v2_mask[sl]
        b1 = np.where(m1, np.float32(-1e30), np.float32(0.0)).astype(np.float32)
        b2 = np.where(m2, np.float32(-1e30), np.float32(0.0)).astype(np.float32)
        bcol = np.ascontiguousarray(b1.reshape(EPC, NB, 128).transpose(2, 0, 1).reshape(128, EPC * NB))
        b2rep = np.repeat(b2[:, None, :], 128, axis=1).reshape(EPC * 128, L)
        k1 = (~m1).astype(np.float32).reshape(EPC, NB, 128).transpose(2, 0, 1).reshape(128, EPC * NB)
        k2 = (~m2).astype(np.float32).reshape(EPC, NB, 128).transpose(2, 0, 1).reshape(128, EPC * NB)
        in_maps.append(
            {
                "v1t": pack_t(v1c).astype(np.float16),
                "v2t": pack_t(v2c).astype(np.float16),
                "v2n": pack_n(v2c).astype(np.float16),
                "v1n": pack_n(v1c).astype(bf16),
                "b2r": np.ascontiguousarray(b2rep),
                "bcol": bcol,
                "ones2": np.ones((128, 2), bf16),
                "cm": np.ascontiguousarray(np.concatenate([k1, k2], axis=1)),
                "idh": np.eye(128, dtype=np.float16),
            }
        )
    return in_maps


def kernel(v1, v2, v1_mask, v2_mask):
    global LAST_RESULTS
    from concourse.bass_utils import run_bass_kernel_spmd

    nc = get_nc()
    in_maps = _host_prep(v1, v2, v1_mask, v2_mask)
    res = run_bass_kernel_spmd(nc, in_maps, list(range(NCORES)))
    LAST_RESULTS = res

    def unpack(name):
        parts = []
        for k in range(NCORES):
            arr = res.results[k][name].astype(np.float32)
            parts.append(arr.reshape(EPC, 128, NB, D).transpose(0, 2, 1, 3).reshape(EPC, L, D))
        return np.concatenate(parts, axis=0)

    return unpack("o1"), unpack("o2")



# revision 29
# speedup vs baseline: 1.0055x; 1.0055x over previous
"""Bidirectional attention kernel for Trainium2 (8 NeuronCores, data-parallel over batch).

Math per example (B=32, L1=L2=512, D=1024):
    sim = v1 @ v2^T                                  [512, 512]
    attn1 = softmax_j(sim + v2maskbias)              (mask v2 cols)
    attn2 = softmax_i(sim + v1maskbias)              (mask v1 rows)
    out1  = (attn1 @ v2) zeroed at v1-masked rows    [512, 1024]
    out2  = (attn2^T @ v1) zeroed at v2-masked rows  [512, 1024]

Device strategy (4 examples per core), all-16-bit datapath:
  - sim matmul operands in fp16 (host pre-transposed); PSUM fp32. fp16
    logits keep softmax ties stable (bf16 does not: 9e-2 rel err).
  - e1 numerators fp16 (range (0,1]); e2 numerators bf16 (range up to
    e^60 from the global-max-bound trick, needs fp32 exponent range).
  - attend rhs: v2 natural fp16 (out1), v1 natural bf16 (out2). Outputs
    bf16, unpacked/upcast on host.
  - Host packs each per-example tensor into a single [128, x] row-major
    region so loads use few, large-descriptor DMAs (DMA issues cost
    ~700ns each on the issuing engine and serialize; the DMA subsystem
    also ramps over the first ~10us of wall clock). All bulk loads ride
    sync's HW DGE queue in strict need order: example 0's sim operands
    in fine chunks (compute tracks the ramp-limited arrivals), example
    1 in quarters, later examples in halves, each example's natural-
    layout attend operands AFTER the next example's sim operands (they
    are consumed one stage later). Scalar's queue carries only small
    early items (bias rows, consts). Example 0's sim runs as two half-K
    passes (partial sums parked in SBUF, bias folded into the first
    add) so all four row blocks compute as soon as chunks 0-3 land.
  - Tail: s2 ones-matmuls + stats hoisted before att1 so the reciprocal
    chain hides under att1; last example's o2 halves store via the idle
    sync/scalar HW queues (gpsimd's software-DGE backlog would gate the
    drain), and the final block's copy splits across DVE+ACT with two
    parallel quarter-stores.
  - e1 -> e1ji transpose on the PE (fp16 transposes run 1 cyc/row, half
    the fp32 cost); PSUM->SBUF copies balanced across ACT and DVE
    (gpsimd/Pool cannot touch PSUM); store issues on gpsimd.
  - Row softmax stats ride the EXP activation accumulator; 1/sum and
    mask-zeroing fold into PSUM->SBUF output copies (per-partition
    scale) which are spread across ACT/DVE/Pool to keep all three off
    the critical path. Column sums for attn2 via tiny ones-matmuls.
  - 1-example software-pipeline skew: sim+softmax of example e issue
    before the attend matmuls of example e-1 so the PE never waits on
    the softmax stats chain and stays at full p-state.
"""

import numpy as np

B, L, D = 32, 512, 1024
NCORES = 8
EPC = B // NCORES  # examples per core
NB = L // 128      # 128-row blocks per L
ND = D // 128      # 128-row chunks per D (transposed layouts)
NDC = D // 512     # 512-col halves per D

_CACHE = {}
LAST_RESULTS = None


def _build_nc():
    from contextlib import ExitStack
    import concourse.bacc as bacc
    import concourse.tile as tile
    import concourse.mybir as mybir
    import concourse.bass_isa as bass_isa

    f32 = mybir.dt.float32
    f16 = mybir.dt.float16
    bf16 = mybir.dt.bfloat16
    EXP = mybir.ActivationFunctionType.Exp
    COPY = mybir.ActivationFunctionType.Copy
    ADD = mybir.AluOpType.add
    MIN = mybir.AluOpType.min
    MAX = mybir.AluOpType.max
    AXX = mybir.AxisListType.X

    nc = bacc.Bacc("TRN2", target_bir_lowering=False, debug=False, num_devices=NCORES)
    # packed layouts: one [128, x] row-major region per example per tensor
    v1td = nc.dram_tensor("v1t", [EPC * 128, ND * L], f16, kind="ExternalInput")
    v2td = nc.dram_tensor("v2t", [EPC * 128, ND * L], f16, kind="ExternalInput")
    v2nd = nc.dram_tensor("v2n", [EPC * 128, NB * D], f16, kind="ExternalInput")
    v1nd = nc.dram_tensor("v1n", [EPC * 128, NB * D], bf16, kind="ExternalInput")
    b2d = nc.dram_tensor("b2r", [EPC * 128, L], f32, kind="ExternalInput")
    cmd = nc.dram_tensor("cm", [128, 2 * EPC * NB], f32, kind="ExternalInput")
    bcd = nc.dram_tensor("bcol", [128, EPC * NB], f32, kind="ExternalInput")
    idd = nc.dram_tensor("idh", [128, 128], f16, kind="ExternalInput")
    ond = nc.dram_tensor("ones2", [128, 2], bf16, kind="ExternalInput")
    o1d = nc.dram_tensor("o1", [EPC * 128, NB * D], bf16, kind="ExternalOutput")
    o2d = nc.dram_tensor("o2", [EPC * 128, NB * D], bf16, kind="ExternalOutput")
    v1ta, v2ta, v2na, v1na = v1td.ap(), v2td.ap(), v2nd.ap(), v1nd.ap()
    o1a, o2a = o1d.ap(), o2d.ap()

    with ExitStack() as ctx:
        # (Tried: pre-TileContext warm-up DMAs to start the DMA ramp early —
        # the ramp turned out to be wall-clock-tied, not first-work-tied, so
        # the warmup only displaced the real loads. Reverted.)
        tc = ctx.enter_context(tile.TileContext(nc))
        const = ctx.enter_context(tc.tile_pool(name="const", bufs=1))
        pv = ctx.enter_context(tc.tile_pool(name="pv", bufs=1))
        pvt = ctx.enter_context(tc.tile_pool(name="pvt", bufs=1))
        pe_ = ctx.enter_context(tc.tile_pool(name="pe", bufs=1))
        pst = ctx.enter_context(tc.tile_pool(name="pst", bufs=1))
        pbb = ctx.enter_context(tc.tile_pool(name="pbb", bufs=1))
        pav = ctx.enter_context(tc.tile_pool(name="pav", bufs=1))
        pps = ctx.enter_context(tc.tile_pool(name="pps", bufs=1, space="PSUM"))

        st = [dict() for _ in range(EPC)]

        def load_warmup():
            # The scalar DMA queue's descriptor pipeline ramps up over ~3us;
            # give it a small const first so b2 (needed ~12us) rides a warm
            # queue. Sync's queue starts directly with the sim-gating chunk
            # (a warmup there just delays the first matmul).
            cms = const.tile([128, 2 * EPC * NB], f32)
            nc.scalar.dma_start(out=cms, in_=cmd.ap())
            return cms

        def load_consts():
            # needed ~15us in (first transpose / stats); after b2 on scalar.
            ident = const.tile([128, 128], f16)
            nc.scalar.dma_start(out=ident, in_=idd.ap())
            bcs = const.tile([128, EPC * NB], f32)
            nc.scalar.dma_start(out=bcs, in_=bcd.ap())
            onesr = const.tile([128, 2], bf16)
            nc.scalar.dma_start(out=onesr, in_=ond.ap())
            return ident, bcs, onesr

        def stage_load(e):
            # DMA-issue instructions cost ~700ns each and serialize per
            # queue; transfers also drain per queue roughly in order while
            # both queues share the 16 DMA engines. So: the sim-critical
            # stream (v1T/v2T interleaved in consumption order, chunked so
            # completion semaphores fire at chunk granularity) gets sync's
            # queue to itself at startup; example 0's small early items
            # (b2, consts) ride scalar's queue.
            s = st[e]
            v1T = pvt.tile([128, ND * L], f16, tag="v1T", bufs=2, name=f"v1T_{e}")
            v2T = pvt.tile([128, ND * L], f16, tag="v2T", bufs=2, name=f"v2T_{e}")
            b2bc0 = pbb.tile([128, L], f32, tag="b2", bufs=2, name=f"b2bc_{e}")
            if e == 0:
                cuts = [0, L, 2 * L, 4 * L, 6 * L, ND * L]
            elif e == 1:
                cuts = [0, 2 * L, 4 * L, 6 * L, ND * L]
            else:
                cuts = [0, 4 * L, ND * L]
            # All bulk loads ride sync's queue in strict need order. (Tried:
            # splitting v1T/v2T across sync+scalar — regresses ~8us because
            # scalar's DMA issues serialize with ACT's exp/copy work
            # mid-body, delaying both the loads and the softmax chain.)
            for a, b in zip(cuts[:-1], cuts[1:]):
                nc.sync.dma_start(out=v1T[:, a:b], in_=v1ta[e * 128 : (e + 1) * 128, a:b])
                nc.sync.dma_start(out=v2T[:, a:b], in_=v2ta[e * 128 : (e + 1) * 128, a:b])
            beng = nc.scalar if e == 0 else nc.sync
            beng.dma_start(out=b2bc0, in_=b2d.ap()[e * 128 : (e + 1) * 128, :])
            s["v1T"] = v1T
            s["v2T"] = v2T
            s["b2bc"] = b2bc0
            # natural-layout attend operands of the PREVIOUS example ride
            # after this example's sim-critical pieces: attend(e-1) runs
            # after sim(e) on the PE, so this ordering matches need order
            # and keeps the DMA-ramp window clear for the sim stream.
            if e > 0:
                stage_load_nat(e - 1)
            if e == EPC - 1:
                stage_load_nat(e)

        def stage_load_nat(e):
            s = st[e]
            v2n = pv.tile([128, NB * D], f16, tag="v2n", bufs=2, name=f"v2n_{e}")
            nc.sync.dma_start(out=v2n, in_=v2na[e * 128 : (e + 1) * 128, :])
            v1n = pv.tile([128, NB * D], bf16, tag="v1n", bufs=2, name=f"v1n_{e}")
            nc.sync.dma_start(out=v1n, in_=v1na[e * 128 : (e + 1) * 128, :])
            s["v2n"] = v2n
            s["v1n"] = v1n

        def stage_sim(e):
            s = st[e]
            m1nt = pst.tile([128, NB], f32, tag="m1nt", bufs=2, name=f"m1nt_{e}")
            s1t = pst.tile([128, NB], f32, tag="s1t", bufs=2, name=f"s1t_{e}")
            s["mk"], s["e1"], s["e2"] = [], [], []
            if e == 0:
                # example 0's operands stream through the cold DMA ramp, so
                # split the K-accumulation in half: all four ib blocks run
                # their first-half chains as soon as chunks 0-3 land (partial
                # sums parked in SBUF with the bias folded in), leaving only
                # the second-half chains gated on the late chunks. (Tried:
                # four concurrent psum chains consuming chunks in arrival
                # order — the tile scheduler reorders the DMA/compute
                # interleave badly and the first matmul slips ~8us.)
                tmpA = []
                for ib in range(NB):
                    psA = pps.tile([128, L], f32, tag="sim", bufs=2)
                    for c in range(ND // 2):
                        nc.tensor.matmul(
                            psA,
                            s["v1T"][:, c * L + ib * 128 : c * L + (ib + 1) * 128],
                            s["v2T"][:, c * L : (c + 1) * L],
                            start=(c == 0),
                            stop=(c == ND // 2 - 1),
                        )
                    ta_ = pe_.tile([128, L], f32, tag="mk", bufs=2 * NB,
                                   name=f"simA_{e}_{ib}")
                    nc.vector.tensor_add(ta_, psA, s["b2bc"])
                    tmpA.append(ta_)
            for ib in range(NB):
                ps = pps.tile([128, L], f32, tag="sim", bufs=2)
                c0 = ND // 2 if e == 0 else 0
                for c in range(c0, ND):
                    nc.tensor.matmul(
                        ps,
                        s["v1T"][:, c * L + ib * 128 : c * L + (ib + 1) * 128],
                        s["v2T"][:, c * L : (c + 1) * L],
                        start=(c == c0),
                        stop=(c == ND - 1),
                    )
                mk = pe_.tile([128, L], f32, tag="mk", bufs=2 * NB, name=f"mk_{e}_{ib}")
                if e == 0:
                    nc.vector.tensor_add(mk, ps, tmpA[ib])
                else:
                    nc.vector.tensor_add(mk, ps, s["b2bc"])
                # m1n = -rowmax(mk): the e1 exp bias, negated in one op
                nc.vector.tensor_reduce(m1nt[:, ib : ib + 1], mk, axis=AXX, op=MAX,
                                        negate=True)
                e1 = pe_.tile([128, L], f16, tag="e1", bufs=2 * NB, name=f"e1_{e}_{ib}")
                nc.scalar.activation(out=e1, in_=mk, func=EXP,
                                     bias=m1nt[:, ib : ib + 1], scale=1.0,
                                     accum_out=s1t[:, ib : ib + 1])
                s["mk"].append(mk)
                s["e1"].append(e1)
            # gm = global max = -min(m1n); all-reduce across partitions
            gmx = pst.tile([128, 1], f32, tag="gmx", bufs=2, name=f"gmx_{e}")
            nc.vector.tensor_reduce(gmx, m1nt, axis=AXX, op=MIN, negate=True)
            gmr = pst.tile([128, 1], f32, tag="gmr", bufs=2, name=f"gmr_{e}")
            nc.gpsimd.partition_all_reduce(gmr, gmx, 128, bass_isa.ReduceOp.max)
            # bias = 60 - gm keeps e2 numerators in normal fp32/bf16 range
            gmn = pst.tile([128, 1], f32, tag="gmn", bufs=2, name=f"gmn_{e}")
            nc.vector.tensor_scalar(gmn, gmr, -1.0, 60.0, op0=mybir.AluOpType.mult,
                                    op1=ADD)
            comb2 = pst.tile([128, NB], f32, tag="comb2", bufs=2, name=f"comb2_{e}")
            nc.vector.tensor_scalar_add(comb2, bcs[:, e * NB : e * NB + NB], gmn)
            r1t = pst.tile([128, NB], f32, tag="r1t", bufs=2, name=f"r1t_{e}")
            nc.vector.reciprocal(out=r1t, in_=s1t)
            sc1t = pst.tile([128, NB], f32, tag="sc1t", bufs=2, name=f"sc1t_{e}")
            nc.vector.tensor_mul(sc1t, r1t, cms[:, e * NB : e * NB + NB])
            s["sc1t"] = sc1t
            # e2 = exp(mk + b1col - gm + 60); b2row term cancels per-column
            for ib in range(NB):
                e2 = pe_.tile([128, L], bf16, tag="e2", bufs=2 * NB, name=f"e2_{e}_{ib}")
                nc.scalar.activation(out=e2, in_=s["mk"][ib], func=EXP,
                                     bias=comb2[:, ib : ib + 1], scale=1.0)
                s["e2"].append(e2)

        def stage_finA(e):
            # transpose e1 into [j,i] lhsT layout (fp16 PE transposes).
            # Emitted BEFORE sim(e+1) so the DVE copies don't queue behind
            # the next example's softmax work (e1 deps are long satisfied).
            s = st[e]
            e1ji = pe_.tile([128, NB * L], f16, tag="e1ji", bufs=2, name=f"e1ji_{e}")
            for jb in range(NB):
                ps = pps.tile([128, L], f16, tag="pte", bufs=2, name=f"pt1_{e}_{jb}")
                for ib in range(NB):
                    nc.tensor.transpose(
                        ps[:, ib * 128 : (ib + 1) * 128],
                        s["e1"][ib][:, jb * 128 : (jb + 1) * 128],
                        ident,
                    )
                # both engines copy half each so the pte PSUM bank frees in
                # half the time (transpose group jb+2 WAR-waits on this copy)
                nc.vector.tensor_copy(e1ji[:, jb * L : jb * L + 256], ps[:, 0:256])
                nc.scalar.copy(e1ji[:, jb * L + 256 : (jb + 1) * L], ps[:, 256:512])
            s["e1ji"] = e1ji

        def stage_finB(e):
            s = st[e]
            e1ji = s["e1ji"]
            # ---- s2 column sums via ones-matmuls on e2 tiles. Hoisted BEFORE
            # att1 so the s2 -> 1/s2 -> sc2t DVE chain hides under att1's
            # ~7us of matmuls instead of gating the att2 output copies.
            pss = pps.tile([128, 2 * NB], f32, tag="pss", bufs=1, name=f"pss_{e}")
            for jb in range(NB):
                for ib in range(NB):
                    nc.tensor.matmul(pss[:, 2 * jb : 2 * jb + 2],
                                     s["e2"][ib][:, jb * 128 : (jb + 1) * 128], onesr,
                                     start=(ib == 0), stop=(ib == NB - 1))
            s2t = pst.tile([128, NB], f32, tag="s2t", bufs=2, name=f"s2t_{e}")
            nc.vector.tensor_scalar_add(s2t, pss[:, 0 : 2 * NB : 2], 1.0e-36)
            r2t = pst.tile([128, NB], f32, tag="r2t", bufs=2, name=f"r2t_{e}")
            nc.vector.reciprocal(out=r2t, in_=s2t)
            sc2t = pst.tile([128, NB], f32, tag="sc2t", bufs=2, name=f"sc2t_{e}")
            nc.vector.tensor_mul(sc2t, r2t, cms[:, EPC * NB + e * NB : EPC * NB + e * NB + NB])
            # ---- out1[i,d] = sum_j e1[j,i] v2[j,d] / s1, masked rows zeroed
            for ib in range(NB):
                av = pav.tile([128, D], bf16, tag="av1", bufs=6)
                for dc in range(NDC):
                    ps = pps.tile([128, 512], f32, tag="att", bufs=3)
                    for jb in range(NB):
                        nc.tensor.matmul(
                            ps,
                            e1ji[:, jb * L + ib * 128 : jb * L + (ib + 1) * 128],
                            s["v2n"][:, jb * D + dc * 512 : jb * D + (dc + 1) * 512],
                            start=(jb == 0),
                            stop=(jb == NB - 1),
                        )
                    dst = av[:, dc * 512 : (dc + 1) * 512]
                    if dc == 0:
                        nc.scalar.activation(out=dst, in_=ps, func=COPY,
                                             scale=s["sc1t"][:, ib : ib + 1])
                    else:
                        nc.vector.tensor_scalar_mul(dst, ps, s["sc1t"][:, ib : ib + 1])
                nc.gpsimd.dma_start(
                    out=o1a[e * 128 : (e + 1) * 128, ib * D : (ib + 1) * D], in_=av)
            # ---- out2[j,d] = sum_i e2[i,j] v1[i,d] / s2, masked rows zeroed
            for jb in range(NB):
                av = pav.tile([128, D], bf16, tag="av2", bufs=6)
                for dc in range(NDC):
                    ps = pps.tile([128, 512], f32, tag="att", bufs=3)
                    for ib in range(NB):
                        nc.tensor.matmul(
                            ps,
                            s["e2"][ib][:, jb * 128 : (jb + 1) * 128],
                            s["v1n"][:, ib * D + dc * 512 : ib * D + (dc + 1) * 512],
                            start=(ib == 0),
                            stop=(ib == NB - 1),
                        )
                    dst = av[:, dc * 512 : (dc + 1) * 512]
                    last = e == EPC - 1 and jb == NB - 1 and dc == NDC - 1
                    if last:
                        # very last block of the kernel: split the copy
                        # across DVE and ACT so the final store chain starts
                        # ~400ns earlier.
                        nc.vector.tensor_scalar_mul(dst[:, 0:256], ps[:, 0:256],
                                                    sc2t[:, jb : jb + 1])
                        nc.scalar.activation(out=dst[:, 256:512], in_=ps[:, 256:512],
                                             func=COPY, scale=sc2t[:, jb : jb + 1])
                    elif dc == 0:
                        nc.vector.tensor_scalar_mul(dst, ps, sc2t[:, jb : jb + 1])
                    else:
                        nc.scalar.activation(out=dst, in_=ps, func=COPY,
                                             scale=sc2t[:, jb : jb + 1])
                    if e == EPC - 1:
                        # kernel-tail drain: store each half as soon as its
                        # copy lands, on the idle hardware DGE queues (sync
                        # for the DVE-copied half, scalar for the ACT-copied
                        # half) so issue+transfer parallelize instead of
                        # queueing behind gpsimd's software-DGE backlog.
                        base = e * 128
                        if last:
                            nc.sync.dma_start(
                                out=o2a[base : base + 128,
                                        jb * D + dc * 512 : jb * D + dc * 512 + 256],
                                in_=dst[:, 0:256])
                            nc.scalar.dma_start(
                                out=o2a[base : base + 128,
                                        jb * D + dc * 512 + 256 : jb * D + (dc + 1) * 512],
                                in_=dst[:, 256:512])
                        else:
                            seng = nc.sync if dc == 0 else nc.scalar
                            seng.dma_start(
                                out=o2a[base : base + 128,
                                        jb * D + dc * 512 : jb * D + (dc + 1) * 512],
                                in_=dst)
                if e != EPC - 1:
                    nc.gpsimd.dma_start(
                        out=o2a[e * 128 : (e + 1) * 128, jb * D : (jb + 1) * D], in_=av)
            st[e] = {}

        cms = load_warmup()
        stage_load(0)
        ident, bcs, onesr = load_consts()
        stage_sim(0)
        for e in range(1, EPC):
            stage_load(e)
            stage_finA(e - 1)
            stage_sim(e)
            stage_finB(e - 1)
        stage_finA(EPC - 1)
        stage_finB(EPC - 1)

    nc.compile()
    return nc


def get_nc():
    if "nc" not in _CACHE:
        _CACHE["nc"] = _build_nc()
    return _CACHE["nc"]


def _host_prep(v1, v2, v1_mask, v2_mask):
    """Build per-core input maps (packed per-example layouts) from full inputs."""
    import ml_dtypes

    bf16 = ml_dtypes.bfloat16
    v1 = np.asarray(v1, dtype=np.float32)
    v2 = np.asarray(v2, dtype=np.float32)
    v1_mask = np.asarray(v1_mask).astype(bool)
    v2_mask = np.asarray(v2_mask).astype(bool)

    def pack_t(x):  # [EPC, L, D] -> [EPC*128, ND*L]; row p = concat chunk rows
        return np.ascontiguousarray(
            x.transpose(0, 2, 1).reshape(EPC, ND, 128, L).transpose(0, 2, 1, 3)
            .reshape(EPC * 128, ND * L))

    def pack_n(x):  # [EPC, L, D] -> [EPC*128, NB*D]; row p = concat block rows
        return np.ascontiguousarray(
            x.reshape(EPC, NB, 128, D).transpose(0, 2, 1, 3).reshape(EPC * 128, NB * D))

    in_maps = []
    for k in range(NCORES):
        sl = slice(EPC * k, EPC * (k + 1))
        v1c, v2c = v1[sl], v2[sl]
        m1 = v1_mask[sl]
        m2 = v2_mask[sl]
        b1 = np.where(m1, np.float32(-1e30), np.float32(0.0)).astype(np.float32)
        b2 = np.where(m2, np.float32(-1e30), np.float32(0.0)).astype(np.float32)
        bcol = np.ascontiguousarray(b1.reshape(EPC, NB, 128).transpose(2, 0, 1).reshape(128, EPC * NB))
        b2rep = np.repeat(b2[:, None, :], 128, axis=1).reshape(EPC * 128, L)
        k1 = (~m1).astype(np.float32).reshape(EPC, NB, 128).transpose(2, 0, 1).reshape(128, EPC * NB)
        k2 = (~m2).astype(np.float32).reshape(EPC, NB, 128).transpose(2, 0, 1).reshape(128, EPC * NB)
        in_maps.append(
            {
                "v1t": pack_t(v1c).astype(np.float16),
                "v2t": pack_t(v2c).astype(np.float16),
                "v2n": pack_n(v2c).astype(np.float16),
                "v1n": pack_n(v1c).astype(bf16),
                "b2r": np.ascontiguousarray(b2rep),
                "bcol": bcol,
                "ones2": np.ones((128, 2), bf16),
                "cm": np.ascontiguousarray(np.concatenate([k1, k2], axis=1)),
                "idh": np.eye(128, dtype=np.float16),
            }
        )
    return in_maps


def kernel(v1, v2, v1_mask, v2_mask):
    global LAST_RESULTS
    from concourse.bass_utils import run_bass_kernel_spmd

    nc = get_nc()
    in_maps = _host_prep(v1, v2, v1_mask, v2_mask)
    res = run_bass_kernel_spmd(nc, in_maps, list(range(NCORES)))
    LAST_RESULTS = res

    def unpack(name):
        parts = []
        for k in range(NCORES):
            arr = res.results[k][name].astype(np.float32)
            parts.append(arr.reshape(EPC, 128, NB, D).transpose(0, 2, 1, 3).reshape(EPC, L, D))
        return np.concatenate(parts, axis=0)

    return unpack("o1"), unpack("o2")



# revision 30
# speedup vs baseline: 1.0168x; 1.0112x over previous
"""Bidirectional attention kernel for Trainium2 (8 NeuronCores, data-parallel over batch).

Math per example (B=32, L1=L2=512, D=1024):
    sim = v1 @ v2^T                                  [512, 512]
    attn1 = softmax_j(sim + v2maskbias)              (mask v2 cols)
    attn2 = softmax_i(sim + v1maskbias)              (mask v1 rows)
    out1  = (attn1 @ v2) zeroed at v1-masked rows    [512, 1024]
    out2  = (attn2^T @ v1) zeroed at v2-masked rows  [512, 1024]

Device strategy (4 examples per core), all-16-bit datapath:
  - sim matmul operands in fp16 (host pre-transposed); PSUM fp32. fp16
    logits keep softmax ties stable (bf16 does not: 9e-2 rel err).
  - e1 numerators fp16 (range (0,1]); e2 numerators bf16 (range up to
    e^60 from the global-max-bound trick, needs fp32 exponent range).
  - attend rhs: v2 natural fp16 (out1), v1 natural bf16 (out2). Outputs
    bf16, unpacked/upcast on host.
  - Host packs each per-example tensor into a single [128, x] row-major
    region so loads use few, large-descriptor DMAs (DMA issues cost
    ~700ns each on the issuing engine and serialize; the DMA subsystem
    also ramps over the first ~10us of wall clock). All bulk loads ride
    sync's HW DGE queue in strict need order: example 0's sim operands
    in fine chunks (compute tracks the ramp-limited arrivals), example
    1 in quarters, later examples in halves, each example's natural-
    layout attend operands AFTER the next example's sim operands (they
    are consumed one stage later). Scalar's queue carries only small
    early items (bias rows, consts). Example 0's sim runs as two half-K
    passes (partial sums parked in SBUF, bias folded into the first
    add) so all four row blocks compute as soon as chunks 0-3 land.
  - Tail: s2 ones-matmuls + stats hoisted before att1 so the reciprocal
    chain hides under att1; last example's o2 halves store via the idle
    sync/scalar HW queues (gpsimd's software-DGE backlog would gate the
    drain), and the final block's copy splits across DVE+ACT with two
    parallel quarter-stores.
  - e1 -> e1ji transpose on the PE (fp16 transposes run 1 cyc/row, half
    the fp32 cost); PSUM->SBUF copies balanced across ACT and DVE
    (gpsimd/Pool cannot touch PSUM); store issues on gpsimd.
  - Row softmax stats ride the EXP activation accumulator; 1/sum and
    mask-zeroing fold into PSUM->SBUF output copies (per-partition
    scale) which are spread across ACT/DVE/Pool to keep all three off
    the critical path. Column sums for attn2 via tiny ones-matmuls.
  - 1-example software-pipeline skew: sim+softmax of example e issue
    before the attend matmuls of example e-1 so the PE never waits on
    the softmax stats chain and stays at full p-state.
"""

import numpy as np

B, L, D = 32, 512, 1024
NCORES = 8
EPC = B // NCORES  # examples per core
NB = L // 128      # 128-row blocks per L
ND = D // 128      # 128-row chunks per D (transposed layouts)
NDC = D // 512     # 512-col halves per D

_CACHE = {}
LAST_RESULTS = None


def _build_nc():
    from contextlib import ExitStack
    import concourse.bacc as bacc
    import concourse.tile as tile
    import concourse.mybir as mybir
    import concourse.bass_isa as bass_isa

    f32 = mybir.dt.float32
    f16 = mybir.dt.float16
    bf16 = mybir.dt.bfloat16
    EXP = mybir.ActivationFunctionType.Exp
    COPY = mybir.ActivationFunctionType.Copy
    ADD = mybir.AluOpType.add
    MIN = mybir.AluOpType.min
    MAX = mybir.AluOpType.max
    AXX = mybir.AxisListType.X

    nc = bacc.Bacc("TRN2", target_bir_lowering=False, debug=False, num_devices=NCORES)
    # packed layouts: one [128, x] row-major region per example per tensor
    v1td = nc.dram_tensor("v1t", [EPC * 128, ND * L], f16, kind="ExternalInput")
    v2td = nc.dram_tensor("v2t", [EPC * 128, ND * L], f16, kind="ExternalInput")
    v2nd = nc.dram_tensor("v2n", [EPC * 128, NB * D], f16, kind="ExternalInput")
    v1nd = nc.dram_tensor("v1n", [EPC * 128, NB * D], bf16, kind="ExternalInput")
    b2d = nc.dram_tensor("b2r", [EPC * 128, L], f32, kind="ExternalInput")
    cmd = nc.dram_tensor("cm", [128, 2 * EPC * NB], f32, kind="ExternalInput")
    bcd = nc.dram_tensor("bcol", [128, EPC * NB], f32, kind="ExternalInput")
    idd = nc.dram_tensor("idh", [128, 128], f16, kind="ExternalInput")
    ond = nc.dram_tensor("ones2", [128, 2], bf16, kind="ExternalInput")
    o1d = nc.dram_tensor("o1", [EPC * 128, NB * D], bf16, kind="ExternalOutput")
    o2d = nc.dram_tensor("o2", [EPC * 128, NB * D], bf16, kind="ExternalOutput")
    v1ta, v2ta, v2na, v1na = v1td.ap(), v2td.ap(), v2nd.ap(), v1nd.ap()
    o1a, o2a = o1d.ap(), o2d.ap()

    with ExitStack() as ctx:
        # (Tried: pre-TileContext warm-up DMAs to start the DMA ramp early —
        # the ramp turned out to be wall-clock-tied, not first-work-tied, so
        # the warmup only displaced the real loads. Reverted.)
        tc = ctx.enter_context(tile.TileContext(nc))
        const = ctx.enter_context(tc.tile_pool(name="const", bufs=1))
        pv = ctx.enter_context(tc.tile_pool(name="pv", bufs=1))
        pvt = ctx.enter_context(tc.tile_pool(name="pvt", bufs=1))
        pe_ = ctx.enter_context(tc.tile_pool(name="pe", bufs=1))
        pst = ctx.enter_context(tc.tile_pool(name="pst", bufs=1))
        pbb = ctx.enter_context(tc.tile_pool(name="pbb", bufs=1))
        pav = ctx.enter_context(tc.tile_pool(name="pav", bufs=1))
        pps = ctx.enter_context(tc.tile_pool(name="pps", bufs=1, space="PSUM"))

        st = [dict() for _ in range(EPC)]

        def load_warmup():
            # The scalar DMA queue's descriptor pipeline ramps up over ~3us;
            # give it a small const first so b2 (needed ~12us) rides a warm
            # queue. Sync's queue starts directly with the sim-gating chunk
            # (a warmup there just delays the first matmul).
            cms = const.tile([128, 2 * EPC * NB], f32)
            nc.scalar.dma_start(out=cms, in_=cmd.ap())
            return cms

        def load_consts():
            # needed ~15us in (first transpose / stats); after b2 on scalar.
            ident = const.tile([128, 128], f16)
            nc.scalar.dma_start(out=ident, in_=idd.ap())
            bcs = const.tile([128, EPC * NB], f32)
            nc.scalar.dma_start(out=bcs, in_=bcd.ap())
            onesr = const.tile([128, 2], bf16)
            nc.scalar.dma_start(out=onesr, in_=ond.ap())
            return ident, bcs, onesr

        def stage_load(e):
            # DMA-issue instructions cost ~700ns each and serialize per
            # queue; transfers also drain per queue roughly in order while
            # both queues share the 16 DMA engines. So: the sim-critical
            # stream (v1T/v2T interleaved in consumption order, chunked so
            # completion semaphores fire at chunk granularity) gets sync's
            # queue to itself at startup; example 0's small early items
            # (b2, consts) ride scalar's queue.
            s = st[e]
            v1T = pvt.tile([128, ND * L], f16, tag="v1T", bufs=2, name=f"v1T_{e}")
            v2T = pvt.tile([128, ND * L], f16, tag="v2T", bufs=2, name=f"v2T_{e}")
            b2bc0 = pbb.tile([128, L], f32, tag="b2", bufs=2, name=f"b2bc_{e}")
            if e == 0:
                cuts = [0, L, 2 * L, 4 * L, 6 * L, ND * L]
            elif e == 1:
                cuts = [0, 2 * L, 4 * L, 6 * L, ND * L]
            else:
                cuts = [0, 4 * L, ND * L]
            # All bulk loads ride sync's queue in strict need order. (Tried:
            # splitting v1T/v2T across sync+scalar — regresses ~8us because
            # scalar's DMA issues serialize with ACT's exp/copy work
            # mid-body, delaying both the loads and the softmax chain.)
            for a, b in zip(cuts[:-1], cuts[1:]):
                nc.sync.dma_start(out=v1T[:, a:b], in_=v1ta[e * 128 : (e + 1) * 128, a:b])
                nc.sync.dma_start(out=v2T[:, a:b], in_=v2ta[e * 128 : (e + 1) * 128, a:b])
            beng = nc.scalar if e == 0 else nc.sync
            beng.dma_start(out=b2bc0, in_=b2d.ap()[e * 128 : (e + 1) * 128, :])
            s["v1T"] = v1T
            s["v2T"] = v2T
            s["b2bc"] = b2bc0
            # natural-layout attend operands of the PREVIOUS example ride
            # after this example's sim-critical pieces: attend(e-1) runs
            # after sim(e) on the PE, so this ordering matches need order
            # and keeps the DMA-ramp window clear for the sim stream.
            if e > 0:
                stage_load_nat(e - 1)
            if e == EPC - 1:
                stage_load_nat(e)

        def stage_load_nat(e):
            s = st[e]
            v2n = pv.tile([128, NB * D], f16, tag="v2n", bufs=2, name=f"v2n_{e}")
            nc.sync.dma_start(out=v2n, in_=v2na[e * 128 : (e + 1) * 128, :])
            v1n = pv.tile([128, NB * D], bf16, tag="v1n", bufs=2, name=f"v1n_{e}")
            nc.sync.dma_start(out=v1n, in_=v1na[e * 128 : (e + 1) * 128, :])
            s["v2n"] = v2n
            s["v1n"] = v1n

        def stage_sim(e):
            s = st[e]
            m1nt = pst.tile([128, NB], f32, tag="m1nt", bufs=2, name=f"m1nt_{e}")
            s1t = pst.tile([128, NB], f32, tag="s1t", bufs=2, name=f"s1t_{e}")
            s["mk"], s["e1"], s["e2"] = [], [], []
            if e == 0:
                # example 0's operands stream through the cold DMA ramp, so
                # split the K-accumulation in half: all four ib blocks run
                # their first-half chains as soon as chunks 0-3 land (partial
                # sums parked in SBUF with the bias folded in), leaving only
                # the second-half chains gated on the late chunks. (Tried:
                # four concurrent psum chains consuming chunks in arrival
                # order — the tile scheduler reorders the DMA/compute
                # interleave badly and the first matmul slips ~8us.)
                tmpA = []
                for ib in range(NB):
                    psA = pps.tile([128, L], f32, tag="sim", bufs=2)
                    for c in range(ND // 2):
                        nc.tensor.matmul(
                            psA,
                            s["v1T"][:, c * L + ib * 128 : c * L + (ib + 1) * 128],
                            s["v2T"][:, c * L : (c + 1) * L],
                            start=(c == 0),
                            stop=(c == ND // 2 - 1),
                        )
                    ta_ = pe_.tile([128, L], f32, tag="mk", bufs=2 * NB,
                                   name=f"simA_{e}_{ib}")
                    nc.vector.tensor_add(ta_, psA, s["b2bc"])
                    tmpA.append(ta_)
            for ib in range(NB):
                if ib < 2:
                    # first two chains borrow the att ring (idle here in
                    # program order): at the e0->sim(1) boundary the sim
                    # ring's banks still WAR-wait e0's mk-adds draining
                    # through the busy DVE (~2us PE stall otherwise).
                    ps = pps.tile([128, 512], f32, tag="att", bufs=3,
                                  name=f"simb_{e}_{ib}")
                else:
                    ps = pps.tile([128, L], f32, tag="sim", bufs=2)
                c0 = ND // 2 if e == 0 else 0
                for c in range(c0, ND):
                    nc.tensor.matmul(
                        ps,
                        s["v1T"][:, c * L + ib * 128 : c * L + (ib + 1) * 128],
                        s["v2T"][:, c * L : (c + 1) * L],
                        start=(c == c0),
                        stop=(c == ND - 1),
                    )
                mk = pe_.tile([128, L], f32, tag="mk", bufs=2 * NB, name=f"mk_{e}_{ib}")
                if e == 0:
                    nc.vector.tensor_add(mk, ps, tmpA[ib])
                else:
                    nc.vector.tensor_add(mk, ps, s["b2bc"])
                # m1n = -rowmax(mk): the e1 exp bias, negated in one op
                nc.vector.tensor_reduce(m1nt[:, ib : ib + 1], mk, axis=AXX, op=MAX,
                                        negate=True)
                e1 = pe_.tile([128, L], f16, tag="e1", bufs=2 * NB, name=f"e1_{e}_{ib}")
                nc.scalar.activation(out=e1, in_=mk, func=EXP,
                                     bias=m1nt[:, ib : ib + 1], scale=1.0,
                                     accum_out=s1t[:, ib : ib + 1])
                s["mk"].append(mk)
                s["e1"].append(e1)
            # gm = global max = -min(m1n); all-reduce across partitions
            gmx = pst.tile([128, 1], f32, tag="gmx", bufs=2, name=f"gmx_{e}")
            nc.vector.tensor_reduce(gmx, m1nt, axis=AXX, op=MIN, negate=True)
            gmr = pst.tile([128, 1], f32, tag="gmr", bufs=2, name=f"gmr_{e}")
            nc.gpsimd.partition_all_reduce(gmr, gmx, 128, bass_isa.ReduceOp.max)
            # bias = 60 - gm keeps e2 numerators in normal fp32/bf16 range
            gmn = pst.tile([128, 1], f32, tag="gmn", bufs=2, name=f"gmn_{e}")
            nc.vector.tensor_scalar(gmn, gmr, -1.0, 60.0, op0=mybir.AluOpType.mult,
                                    op1=ADD)
            comb2 = pst.tile([128, NB], f32, tag="comb2", bufs=2, name=f"comb2_{e}")
            nc.vector.tensor_scalar_add(comb2, bcs[:, e * NB : e * NB + NB], gmn)
            r1t = pst.tile([128, NB], f32, tag="r1t", bufs=2, name=f"r1t_{e}")
            nc.vector.reciprocal(out=r1t, in_=s1t)
            sc1t = pst.tile([128, NB], f32, tag="sc1t", bufs=2, name=f"sc1t_{e}")
            nc.vector.tensor_mul(sc1t, r1t, cms[:, e * NB : e * NB + NB])
            s["sc1t"] = sc1t
            # e2 = exp(mk + b1col - gm + 60); b2row term cancels per-column
            for ib in range(NB):
                e2 = pe_.tile([128, L], bf16, tag="e2", bufs=2 * NB, name=f"e2_{e}_{ib}")
                nc.scalar.activation(out=e2, in_=s["mk"][ib], func=EXP,
                                     bias=comb2[:, ib : ib + 1], scale=1.0)
                s["e2"].append(e2)

        def stage_finA(e):
            # transpose e1 into [j,i] lhsT layout (fp16 PE transposes).
            # Emitted BEFORE sim(e+1) so the DVE copies don't queue behind
            # the next example's softmax work (e1 deps are long satisfied).
            s = st[e]
            e1ji = pe_.tile([128, NB * L], f16, tag="e1ji", bufs=2, name=f"e1ji_{e}")
            for jb in range(NB):
                ps = pps.tile([128, L], f16, tag="pte", bufs=2, name=f"pt1_{e}_{jb}")
                for ib in range(NB):
                    nc.tensor.transpose(
                        ps[:, ib * 128 : (ib + 1) * 128],
                        s["e1"][ib][:, jb * 128 : (jb + 1) * 128],
                        ident,
                    )
                # both engines copy half each so the pte PSUM bank frees in
                # half the time (transpose group jb+2 WAR-waits on this copy)
                nc.vector.tensor_copy(e1ji[:, jb * L : jb * L + 256], ps[:, 0:256])
                nc.scalar.copy(e1ji[:, jb * L + 256 : (jb + 1) * L], ps[:, 256:512])
            s["e1ji"] = e1ji

        def stage_finB(e):
            s = st[e]
            e1ji = s["e1ji"]
            # ---- s2 column sums via ones-matmuls on e2 tiles. Hoisted BEFORE
            # att1 so the s2 -> 1/s2 -> sc2t DVE chain hides under att1's
            # ~7us of matmuls instead of gating the att2 output copies.
            pss = pps.tile([128, 2 * NB], f32, tag="pss", bufs=1, name=f"pss_{e}")
            for jb in range(NB):
                for ib in range(NB):
                    nc.tensor.matmul(pss[:, 2 * jb : 2 * jb + 2],
                                     s["e2"][ib][:, jb * 128 : (jb + 1) * 128], onesr,
                                     start=(ib == 0), stop=(ib == NB - 1))
            s2t = pst.tile([128, NB], f32, tag="s2t", bufs=2, name=f"s2t_{e}")
            nc.vector.tensor_scalar_add(s2t, pss[:, 0 : 2 * NB : 2], 1.0e-36)
            r2t = pst.tile([128, NB], f32, tag="r2t", bufs=2, name=f"r2t_{e}")
            nc.vector.reciprocal(out=r2t, in_=s2t)
            sc2t = pst.tile([128, NB], f32, tag="sc2t", bufs=2, name=f"sc2t_{e}")
            nc.vector.tensor_mul(sc2t, r2t, cms[:, EPC * NB + e * NB : EPC * NB + e * NB + NB])
            # ---- out1[i,d] = sum_j e1[j,i] v2[j,d] / s1, masked rows zeroed
            for ib in range(NB):
                av = pav.tile([128, D], bf16, tag="av1", bufs=6)
                for dc in range(NDC):
                    ps = pps.tile([128, 512], f32, tag="att", bufs=3)
                    for jb in range(NB):
                        nc.tensor.matmul(
                            ps,
                            e1ji[:, jb * L + ib * 128 : jb * L + (ib + 1) * 128],
                            s["v2n"][:, jb * D + dc * 512 : jb * D + (dc + 1) * 512],
                            start=(jb == 0),
                            stop=(jb == NB - 1),
                        )
                    dst = av[:, dc * 512 : (dc + 1) * 512]
                    if dc == 0:
                        nc.scalar.activation(out=dst, in_=ps, func=COPY,
                                             scale=s["sc1t"][:, ib : ib + 1])
                    else:
                        nc.vector.tensor_scalar_mul(dst, ps, s["sc1t"][:, ib : ib + 1])
                nc.gpsimd.dma_start(
                    out=o1a[e * 128 : (e + 1) * 128, ib * D : (ib + 1) * D], in_=av)
            # ---- out2[j,d] = sum_i e2[i,j] v1[i,d] / s2, masked rows zeroed
            for jb in range(NB):
                av = pav.tile([128, D], bf16, tag="av2", bufs=6)
                for dc in range(NDC):
                    ps = pps.tile([128, 512], f32, tag="att", bufs=3)
                    for ib in range(NB):
                        nc.tensor.matmul(
                            ps,
                            s["e2"][ib][:, jb * 128 : (jb + 1) * 128],
                            s["v1n"][:, ib * D + dc * 512 : ib * D + (dc + 1) * 512],
                            start=(ib == 0),
                            stop=(ib == NB - 1),
                        )
                    dst = av[:, dc * 512 : (dc + 1) * 512]
                    last = e == EPC - 1 and jb == NB - 1 and dc == NDC - 1
                    if last:
                        # very last block of the kernel: split the copy
                        # across DVE and ACT so the final store chain starts
                        # ~400ns earlier.
                        nc.vector.tensor_scalar_mul(dst[:, 0:256], ps[:, 0:256],
                                                    sc2t[:, jb : jb + 1])
                        nc.scalar.activation(out=dst[:, 256:512], in_=ps[:, 256:512],
                                             func=COPY, scale=sc2t[:, jb : jb + 1])
                    elif dc == 0:
                        nc.vector.tensor_scalar_mul(dst, ps, sc2t[:, jb : jb + 1])
                    else:
                        nc.scalar.activation(out=dst, in_=ps, func=COPY,
                                             scale=sc2t[:, jb : jb + 1])
                    if e == EPC - 1:
                        # kernel-tail drain: store each half as soon as its
                        # copy lands, on the idle hardware DGE queues (sync
                        # for the DVE-copied half, scalar for the ACT-copied
                        # half) so issue+transfer parallelize instead of
                        # queueing behind gpsimd's software-DGE backlog.
                        base = e * 128
                        if last:
                            nc.sync.dma_start(
                                out=o2a[base : base + 128,
                                        jb * D + dc * 512 : jb * D + dc * 512 + 256],
                                in_=dst[:, 0:256])
                            nc.scalar.dma_start(
                                out=o2a[base : base + 128,
                                        jb * D + dc * 512 + 256 : jb * D + (dc + 1) * 512],
                                in_=dst[:, 256:512])
                        else:
                            seng = nc.sync if dc == 0 else nc.scalar
                            seng.dma_start(
                                out=o2a[base : base + 128,
                                        jb * D + dc * 512 : jb * D + (dc + 1) * 512],
                                in_=dst)
                if e != EPC - 1:
                    nc.gpsimd.dma_start(
                        out=o2a[e * 128 : (e + 1) * 128, jb * D : (jb + 1) * D], in_=av)
            st[e] = {}

        cms = load_warmup()
        stage_load(0)
        ident, bcs, onesr = load_consts()
        stage_sim(0)
        for e in range(1, EPC):
            stage_load(e)
            stage_finA(e - 1)
            stage_sim(e)
            stage_finB(e - 1)
        stage_finA(EPC - 1)
        stage_finB(EPC - 1)

    nc.compile()
    return nc


def get_nc():
    if "nc" not in _CACHE:
        _CACHE["nc"] = _build_nc()
    return _CACHE["nc"]


def _host_prep(v1, v2, v1_mask, v2_mask):
    """Build per-core input maps (packed per-example layouts) from full inputs."""
    import ml_dtypes

    bf16 = ml_dtypes.bfloat16
    v1 = np.asarray(v1, dtype=np.float32)
    v2 = np.asarray(v2, dtype=np.float32)
    v1_mask = np.asarray(v1_mask).astype(bool)
    v2_mask = np.asarray(v2_mask).astype(bool)

    def pack_t(x):  # [EPC, L, D] -> [EPC*128, ND*L]; row p = concat chunk rows
        return np.ascontiguousarray(
            x.transpose(0, 2, 1).reshape(EPC, ND, 128, L).transpose(0, 2, 1, 3)
            .reshape(EPC * 128, ND * L))

    def pack_n(x):  # [EPC, L, D] -> [EPC*128, NB*D]; row p = concat block rows
        return np.ascontiguousarray(
            x.reshape(EPC, NB, 128, D).transpose(0, 2, 1, 3).reshape(EPC * 128, NB * D))

    in_maps = []
    for k in range(NCORES):
        sl = slice(EPC * k, EPC * (k + 1))
        v1c, v2c = v1[sl], v2[sl]
        m1 = v1_mask[sl]
        m2 = v2_mask[sl]
        b1 = np.where(m1, np.float32(-1e30), np.float32(0.0)).astype(np.float32)
        b2 = np.where(m2, np.float32(-1e30), np.float32(0.0)).astype(np.float32)
        bcol = np.ascontiguousarray(b1.reshape(EPC, NB, 128).transpose(2, 0, 1).reshape(128, EPC * NB))
        b2rep = np.repeat(b2[:, None, :], 128, axis=1).reshape(EPC * 128, L)
        k1 = (~m1).astype(np.float32).reshape(EPC, NB, 128).transpose(2, 0, 1).reshape(128, EPC * NB)
        k2 = (~m2).astype(np.float32).reshape(EPC, NB, 128).transpose(2, 0, 1).reshape(128, EPC * NB)
        in_maps.append(
            {
                "v1t": pack_t(v1c).astype(np.float16),
                "v2t": pack_t(v2c).astype(np.float16),
                "v2n": pack_n(v2c).astype(np.float16),
                "v1n": pack_n(v1c).astype(bf16),
                "b2r": np.ascontiguousarray(b2rep),
                "bcol": bcol,
                "ones2": np.ones((128, 2), bf16),
                "cm": np.ascontiguousarray(np.concatenate([k1, k2], axis=1)),
                "idh": np.eye(128, dtype=np.float16),
            }
        )
    return in_maps


def kernel(v1, v2, v1_mask, v2_mask):
    global LAST_RESULTS
    from concourse.bass_utils import run_bass_kernel_spmd

    nc = get_nc()
    in_maps = _host_prep(v1, v2, v1_mask, v2_mask)
    res = run_bass_kernel_spmd(nc, in_maps, list(range(NCORES)))
    LAST_RESULTS = res

    def unpack(name):
        parts = []
        for k in range(NCORES):
            arr = res.results[k][name].astype(np.float32)
            parts.append(arr.reshape(EPC, 128, NB, D).transpose(0, 2, 1, 3).reshape(EPC, L, D))
        return np.concatenate(parts, axis=0)

    return unpack("o1"), unpack("o2")



# revision 32
# speedup vs baseline: 1.0308x; 1.0138x over previous
"""Bidirectional attention kernel for Trainium2 (8 NeuronCores, data-parallel over batch).

Math per example (B=32, L1=L2=512, D=1024):
    sim = v1 @ v2^T                                  [512, 512]
    attn1 = softmax_j(sim + v2maskbias)              (mask v2 cols)
    attn2 = softmax_i(sim + v1maskbias)              (mask v1 rows)
    out1  = (attn1 @ v2) zeroed at v1-masked rows    [512, 1024]
    out2  = (attn2^T @ v1) zeroed at v2-masked rows  [512, 1024]

Device strategy (4 examples per core), all-16-bit datapath:
  - sim matmul operands in fp16 (host pre-transposed); PSUM fp32. fp16
    logits keep softmax ties stable (bf16 does not: 9e-2 rel err).
  - e1 numerators fp16 (range (0,1]); e2 numerators bf16 (range up to
    e^60 from the global-max-bound trick, needs fp32 exponent range).
  - attend rhs: v2 natural fp16 (out1), v1 natural bf16 (out2). Outputs
    bf16, unpacked/upcast on host.
  - Host packs each per-example tensor into a single [128, x] row-major
    region so loads use few, large-descriptor DMAs (DMA issues cost
    ~700ns each on the issuing engine and serialize; the DMA subsystem
    also ramps over the first ~10us of wall clock). All bulk loads ride
    sync's HW DGE queue in strict need order: example 0's sim operands
    in fine chunks (compute tracks the ramp-limited arrivals), example
    1 in quarters, later examples in halves, each example's natural-
    layout attend operands AFTER the next example's sim operands (they
    are consumed one stage later). Scalar's queue carries only small
    early items (bias rows, consts). Example 0's sim runs as two half-K
    passes (partial sums parked in SBUF, bias folded into the first
    add) so all four row blocks compute as soon as chunks 0-3 land.
  - Tail: s2 ones-matmuls + stats hoisted before att1 so the reciprocal
    chain hides under att1; last example's o2 halves store via the idle
    sync/scalar HW queues (gpsimd's software-DGE backlog would gate the
    drain), and the final block's copy splits across DVE+ACT with two
    parallel quarter-stores.
  - e1 -> e1ji transpose on the PE (fp16 transposes run 1 cyc/row, half
    the fp32 cost); PSUM->SBUF copies balanced across ACT and DVE
    (gpsimd/Pool cannot touch PSUM); store issues on gpsimd.
  - Row softmax stats ride the EXP activation accumulator; 1/sum and
    mask-zeroing fold into PSUM->SBUF output copies (per-partition
    scale) which are spread across ACT/DVE/Pool to keep all three off
    the critical path. Column sums for attn2 via tiny ones-matmuls.
  - 1-example software-pipeline skew: sim+softmax of example e issue
    before the attend matmuls of example e-1 so the PE never waits on
    the softmax stats chain and stays at full p-state.
"""

import numpy as np

B, L, D = 32, 512, 1024
NCORES = 8
EPC = B // NCORES  # examples per core
NB = L // 128      # 128-row blocks per L
ND = D // 128      # 128-row chunks per D (transposed layouts)
NDC = D // 512     # 512-col halves per D

_CACHE = {}
LAST_RESULTS = None


def _build_nc():
    from contextlib import ExitStack
    import concourse.bacc as bacc
    import concourse.tile as tile
    import concourse.mybir as mybir
    import concourse.bass_isa as bass_isa

    f32 = mybir.dt.float32
    f16 = mybir.dt.float16
    bf16 = mybir.dt.bfloat16
    EXP = mybir.ActivationFunctionType.Exp
    COPY = mybir.ActivationFunctionType.Copy
    ADD = mybir.AluOpType.add
    MIN = mybir.AluOpType.min
    MAX = mybir.AluOpType.max
    AXX = mybir.AxisListType.X

    nc = bacc.Bacc("TRN2", target_bir_lowering=False, debug=False, num_devices=NCORES)
    # packed layouts: one [128, x] row-major region per example per tensor
    v1td = nc.dram_tensor("v1t", [EPC * 128, ND * L], f16, kind="ExternalInput")
    v2td = nc.dram_tensor("v2t", [EPC * 128, ND * L], f16, kind="ExternalInput")
    v2nd = nc.dram_tensor("v2n", [EPC * 128, NB * D], f16, kind="ExternalInput")
    v1nd = nc.dram_tensor("v1n", [EPC * 128, NB * D], bf16, kind="ExternalInput")
    b2d = nc.dram_tensor("b2r", [EPC * 128, L], f32, kind="ExternalInput")
    cmd = nc.dram_tensor("cm", [128, 2 * EPC * NB], f32, kind="ExternalInput")
    bcd = nc.dram_tensor("bcol", [128, EPC * NB], f32, kind="ExternalInput")
    idd = nc.dram_tensor("idh", [128, 128], f16, kind="ExternalInput")
    ond = nc.dram_tensor("ones2", [128, 2], bf16, kind="ExternalInput")
    o1d = nc.dram_tensor("o1", [EPC * 128, NB * D], bf16, kind="ExternalOutput")
    o2d = nc.dram_tensor("o2", [EPC * 128, NB * D], bf16, kind="ExternalOutput")
    v1ta, v2ta, v2na, v1na = v1td.ap(), v2td.ap(), v2nd.ap(), v1nd.ap()
    o1a, o2a = o1d.ap(), o2d.ap()

    with ExitStack() as ctx:
        # (Tried: pre-TileContext warm-up DMAs to start the DMA ramp early —
        # the ramp turned out to be wall-clock-tied, not first-work-tied, so
        # the warmup only displaced the real loads. Reverted.)
        tc = ctx.enter_context(tile.TileContext(nc))
        const = ctx.enter_context(tc.tile_pool(name="const", bufs=1))
        pv = ctx.enter_context(tc.tile_pool(name="pv", bufs=1))
        pvt = ctx.enter_context(tc.tile_pool(name="pvt", bufs=1))
        pe_ = ctx.enter_context(tc.tile_pool(name="pe", bufs=1))
        pst = ctx.enter_context(tc.tile_pool(name="pst", bufs=1))
        pbb = ctx.enter_context(tc.tile_pool(name="pbb", bufs=1))
        pav = ctx.enter_context(tc.tile_pool(name="pav", bufs=1))
        pps = ctx.enter_context(tc.tile_pool(name="pps", bufs=1, space="PSUM"))

        st = [dict() for _ in range(EPC)]

        def load_warmup():
            # ident rides scalar's queue FIRST (lands ~9.8us, well before its
            # real first use at finA(0)); it doubles as the warm-queue fodder
            # for b2 AND as the operand for the PE clock warm-up below.
            # Sync's queue starts directly with the sim-gating chunk (a
            # warmup there just delays the first matmul).
            ident = const.tile([128, 128], f16)
            nc.scalar.dma_start(out=ident, in_=idd.ap())
            cms = const.tile([128, 2 * EPC * NB], f32)
            nc.scalar.dma_start(out=cms, in_=cmd.ap())
            return ident, cms

        def pe_clock_warmup(ident):
            # The PE clock ramps only while the engine executes (observed:
            # early matmuls at 585-634ns vs 379ns warm). Fill the otherwise
            # idle ~10-12.5us window (waiting the first sim chunks) with
            # never-read dummy matmuls on ident so the real sim starts on a
            # ramped clock. Output goes to an att-ring psum slice that has
            # no reader; its WAR resolves long before the first attend.
            dps = pps.tile([128, 512], f32, tag="att", bufs=3, name="pewarm")
            for i in range(16):
                nc.tensor.matmul(dps[:, 0:128], ident, ident,
                                 start=True, stop=True)

        def load_consts():
            # needed ~15us in (stats / finB); after b2 on scalar.
            bcs = const.tile([128, EPC * NB], f32)
            nc.scalar.dma_start(out=bcs, in_=bcd.ap())
            onesr = const.tile([128, 2], bf16)
            nc.scalar.dma_start(out=onesr, in_=ond.ap())
            return bcs, onesr

        def stage_load(e):
            # DMA-issue instructions cost ~700ns each and serialize per
            # queue; transfers also drain per queue roughly in order while
            # both queues share the 16 DMA engines. So: the sim-critical
            # stream (v1T/v2T interleaved in consumption order, chunked so
            # completion semaphores fire at chunk granularity) gets sync's
            # queue to itself at startup; example 0's small early items
            # (b2, consts) ride scalar's queue.
            s = st[e]
            v1T = pvt.tile([128, ND * L], f16, tag="v1T", bufs=2, name=f"v1T_{e}")
            v2T = pvt.tile([128, ND * L], f16, tag="v2T", bufs=2, name=f"v2T_{e}")
            b2bc0 = pbb.tile([128, L], f32, tag="b2", bufs=2, name=f"b2bc_{e}")
            if e == 0:
                cuts = [0, L, 2 * L, 4 * L, 6 * L, ND * L]
            elif e == 1:
                cuts = [0, 2 * L, 4 * L, 6 * L, ND * L]
            else:
                cuts = [0, 4 * L, ND * L]
            # All bulk loads ride sync's queue in strict need order. (Tried:
            # splitting v1T/v2T across sync+scalar — regresses ~8us because
            # scalar's DMA issues serialize with ACT's exp/copy work
            # mid-body, delaying both the loads and the softmax chain.)
            for a, b in zip(cuts[:-1], cuts[1:]):
                nc.sync.dma_start(out=v1T[:, a:b], in_=v1ta[e * 128 : (e + 1) * 128, a:b])
                nc.sync.dma_start(out=v2T[:, a:b], in_=v2ta[e * 128 : (e + 1) * 128, a:b])
            beng = nc.scalar if e == 0 else nc.sync
            beng.dma_start(out=b2bc0, in_=b2d.ap()[e * 128 : (e + 1) * 128, :])
            s["v1T"] = v1T
            s["v2T"] = v2T
            s["b2bc"] = b2bc0
            # natural-layout attend operands of the PREVIOUS example ride
            # after this example's sim-critical pieces: attend(e-1) runs
            # after sim(e) on the PE, so this ordering matches need order
            # and keeps the DMA-ramp window clear for the sim stream.
            if e > 0:
                stage_load_nat(e - 1)
            if e == EPC - 1:
                stage_load_nat(e)

        def stage_load_nat(e):
            s = st[e]
            v2n = pv.tile([128, NB * D], f16, tag="v2n", bufs=2, name=f"v2n_{e}")
            nc.sync.dma_start(out=v2n, in_=v2na[e * 128 : (e + 1) * 128, :])
            v1n = pv.tile([128, NB * D], bf16, tag="v1n", bufs=2, name=f"v1n_{e}")
            nc.sync.dma_start(out=v1n, in_=v1na[e * 128 : (e + 1) * 128, :])
            s["v2n"] = v2n
            s["v1n"] = v1n

        def stage_sim(e):
            s = st[e]
            m1nt = pst.tile([128, NB], f32, tag="m1nt", bufs=2, name=f"m1nt_{e}")
            s1t = pst.tile([128, NB], f32, tag="s1t", bufs=2, name=f"s1t_{e}")
            s["mk"], s["e1"], s["e2"] = [], [], []
            if e == 0:
                # example 0's operands stream through the cold DMA ramp, so
                # split the K-accumulation in half: all four ib blocks run
                # their first-half chains as soon as chunks 0-3 land (partial
                # sums parked in SBUF with the bias folded in), leaving only
                # the second-half chains gated on the late chunks. (Tried:
                # four concurrent psum chains consuming chunks in arrival
                # order — the tile scheduler reorders the DMA/compute
                # interleave badly and the first matmul slips ~8us.)
                tmpA = []
                for ib in range(NB):
                    psA = pps.tile([128, L], f32, tag="sim", bufs=2)
                    for c in range(ND // 2):
                        nc.tensor.matmul(
                            psA,
                            s["v1T"][:, c * L + ib * 128 : c * L + (ib + 1) * 128],
                            s["v2T"][:, c * L : (c + 1) * L],
                            start=(c == 0),
                            stop=(c == ND // 2 - 1),
                        )
                    ta_ = pe_.tile([128, L], f32, tag="mk", bufs=2 * NB,
                                   name=f"simA_{e}_{ib}")
                    nc.vector.tensor_add(ta_, psA, s["b2bc"])
                    tmpA.append(ta_)
            for ib in range(NB):
                if ib < 2:
                    # first two chains borrow the att ring (idle here in
                    # program order): at the e0->sim(1) boundary the sim
                    # ring's banks still WAR-wait e0's mk-adds draining
                    # through the busy DVE (~2us PE stall otherwise).
                    ps = pps.tile([128, 512], f32, tag="att", bufs=3,
                                  name=f"simb_{e}_{ib}")
                else:
                    ps = pps.tile([128, L], f32, tag="sim", bufs=2)
                c0 = ND // 2 if e == 0 else 0
                for c in range(c0, ND):
                    nc.tensor.matmul(
                        ps,
                        s["v1T"][:, c * L + ib * 128 : c * L + (ib + 1) * 128],
                        s["v2T"][:, c * L : (c + 1) * L],
                        start=(c == c0),
                        stop=(c == ND - 1),
                    )
                mk = pe_.tile([128, L], f32, tag="mk", bufs=2 * NB, name=f"mk_{e}_{ib}")
                if e == 0:
                    nc.vector.tensor_add(mk, ps, tmpA[ib])
                else:
                    nc.vector.tensor_add(mk, ps, s["b2bc"])
                # m1n = -rowmax(mk): the e1 exp bias, negated in one op
                nc.vector.tensor_reduce(m1nt[:, ib : ib + 1], mk, axis=AXX, op=MAX,
                                        negate=True)
                e1 = pe_.tile([128, L], f16, tag="e1", bufs=2 * NB, name=f"e1_{e}_{ib}")
                nc.scalar.activation(out=e1, in_=mk, func=EXP,
                                     bias=m1nt[:, ib : ib + 1], scale=1.0,
                                     accum_out=s1t[:, ib : ib + 1])
                s["mk"].append(mk)
                s["e1"].append(e1)
            # gm = global max = -min(m1n); all-reduce across partitions
            gmx = pst.tile([128, 1], f32, tag="gmx", bufs=2, name=f"gmx_{e}")
            nc.vector.tensor_reduce(gmx, m1nt, axis=AXX, op=MIN, negate=True)
            gmr = pst.tile([128, 1], f32, tag="gmr", bufs=2, name=f"gmr_{e}")
            nc.gpsimd.partition_all_reduce(gmr, gmx, 128, bass_isa.ReduceOp.max)
            # bias = 60 - gm keeps e2 numerators in normal fp32/bf16 range
            gmn = pst.tile([128, 1], f32, tag="gmn", bufs=2, name=f"gmn_{e}")
            nc.vector.tensor_scalar(gmn, gmr, -1.0, 60.0, op0=mybir.AluOpType.mult,
                                    op1=ADD)
            comb2 = pst.tile([128, NB], f32, tag="comb2", bufs=2, name=f"comb2_{e}")
            nc.vector.tensor_scalar_add(comb2, bcs[:, e * NB : e * NB + NB], gmn)
            r1t = pst.tile([128, NB], f32, tag="r1t", bufs=2, name=f"r1t_{e}")
            nc.vector.reciprocal(out=r1t, in_=s1t)
            sc1t = pst.tile([128, NB], f32, tag="sc1t", bufs=2, name=f"sc1t_{e}")
            nc.vector.tensor_mul(sc1t, r1t, cms[:, e * NB : e * NB + NB])
            s["sc1t"] = sc1t
            # e2 = exp(mk + b1col - gm + 60); b2row term cancels per-column
            for ib in range(NB):
                e2 = pe_.tile([128, L], bf16, tag="e2", bufs=2 * NB, name=f"e2_{e}_{ib}")
                nc.scalar.activation(out=e2, in_=s["mk"][ib], func=EXP,
                                     bias=comb2[:, ib : ib + 1], scale=1.0)
                s["e2"].append(e2)

        def stage_finA(e):
            # transpose e1 into [j,i] lhsT layout (fp16 PE transposes).
            # Emitted BEFORE sim(e+1) so the DVE copies don't queue behind
            # the next example's softmax work (e1 deps are long satisfied).
            s = st[e]
            e1ji = pe_.tile([128, NB * L], f16, tag="e1ji", bufs=2, name=f"e1ji_{e}")
            for jb in range(NB):
                ps = pps.tile([128, L], f16, tag="pte", bufs=2, name=f"pt1_{e}_{jb}")
                for ib in range(NB):
                    nc.tensor.transpose(
                        ps[:, ib * 128 : (ib + 1) * 128],
                        s["e1"][ib][:, jb * 128 : (jb + 1) * 128],
                        ident,
                    )
                # both engines copy half each so the pte PSUM bank frees in
                # half the time (transpose group jb+2 WAR-waits on this copy)
                nc.vector.tensor_copy(e1ji[:, jb * L : jb * L + 256], ps[:, 0:256])
                nc.scalar.copy(e1ji[:, jb * L + 256 : (jb + 1) * L], ps[:, 256:512])
            s["e1ji"] = e1ji

        def stage_finB(e):
            s = st[e]
            e1ji = s["e1ji"]
            # ---- s2 column sums via ones-matmuls on e2 tiles. Hoisted BEFORE
            # att1 so the s2 -> 1/s2 -> sc2t DVE chain hides under att1's
            # ~7us of matmuls instead of gating the att2 output copies.
            pss = pps.tile([128, 2 * NB], f32, tag="pss", bufs=1, name=f"pss_{e}")
            for jb in range(NB):
                for ib in range(NB):
                    nc.tensor.matmul(pss[:, 2 * jb : 2 * jb + 2],
                                     s["e2"][ib][:, jb * 128 : (jb + 1) * 128], onesr,
                                     start=(ib == 0), stop=(ib == NB - 1))
            s2t = pst.tile([128, NB], f32, tag="s2t", bufs=2, name=f"s2t_{e}")
            nc.vector.tensor_scalar_add(s2t, pss[:, 0 : 2 * NB : 2], 1.0e-36)
            r2t = pst.tile([128, NB], f32, tag="r2t", bufs=2, name=f"r2t_{e}")
            nc.vector.reciprocal(out=r2t, in_=s2t)
            sc2t = pst.tile([128, NB], f32, tag="sc2t", bufs=2, name=f"sc2t_{e}")
            nc.vector.tensor_mul(sc2t, r2t, cms[:, EPC * NB + e * NB : EPC * NB + e * NB + NB])
            # ---- out1[i,d] = sum_j e1[j,i] v2[j,d] / s1, masked rows zeroed
            for ib in range(NB):
                av = pav.tile([128, D], bf16, tag="av1", bufs=6)
                for dc in range(NDC):
                    ps = pps.tile([128, 512], f32, tag="att", bufs=3)
                    for jb in range(NB):
                        nc.tensor.matmul(
                            ps,
                            e1ji[:, jb * L + ib * 128 : jb * L + (ib + 1) * 128],
                            s["v2n"][:, jb * D + dc * 512 : jb * D + (dc + 1) * 512],
                            start=(jb == 0),
                            stop=(jb == NB - 1),
                        )
                    dst = av[:, dc * 512 : (dc + 1) * 512]
                    if dc == 0:
                        nc.scalar.activation(out=dst, in_=ps, func=COPY,
                                             scale=s["sc1t"][:, ib : ib + 1])
                    else:
                        nc.vector.tensor_scalar_mul(dst, ps, s["sc1t"][:, ib : ib + 1])
                nc.gpsimd.dma_start(
                    out=o1a[e * 128 : (e + 1) * 128, ib * D : (ib + 1) * D], in_=av)
            # ---- out2[j,d] = sum_i e2[i,j] v1[i,d] / s2, masked rows zeroed
            for jb in range(NB):
                av = pav.tile([128, D], bf16, tag="av2", bufs=6)
                for dc in range(NDC):
                    ps = pps.tile([128, 512], f32, tag="att", bufs=3)
                    for ib in range(NB):
                        nc.tensor.matmul(
                            ps,
                            s["e2"][ib][:, jb * 128 : (jb + 1) * 128],
                            s["v1n"][:, ib * D + dc * 512 : ib * D + (dc + 1) * 512],
                            start=(ib == 0),
                            stop=(ib == NB - 1),
                        )
                    dst = av[:, dc * 512 : (dc + 1) * 512]
                    last = e == EPC - 1 and jb == NB - 1 and dc == NDC - 1
                    if last:
                        # very last block of the kernel: split the copy
                        # across DVE and ACT so the final store chain starts
                        # ~400ns earlier.
                        nc.vector.tensor_scalar_mul(dst[:, 0:256], ps[:, 0:256],
                                                    sc2t[:, jb : jb + 1])
                        nc.scalar.activation(out=dst[:, 256:512], in_=ps[:, 256:512],
                                             func=COPY, scale=sc2t[:, jb : jb + 1])
                    elif dc == 0:
                        nc.vector.tensor_scalar_mul(dst, ps, sc2t[:, jb : jb + 1])
                    else:
                        nc.scalar.activation(out=dst, in_=ps, func=COPY,
                                             scale=sc2t[:, jb : jb + 1])
                    if e == EPC - 1:
                        # kernel-tail drain: store each half as soon as its
                        # copy lands, on the idle hardware DGE queues (sync
                        # for the DVE-copied half, scalar for the ACT-copied
                        # half) so issue+transfer parallelize instead of
                        # queueing behind gpsimd's software-DGE backlog.
                        base = e * 128
                        if last:
                            nc.sync.dma_start(
                                out=o2a[base : base + 128,
                                        jb * D + dc * 512 : jb * D + dc * 512 + 256],
                                in_=dst[:, 0:256])
                            nc.scalar.dma_start(
                                out=o2a[base : base + 128,
                                        jb * D + dc * 512 + 256 : jb * D + (dc + 1) * 512],
                                in_=dst[:, 256:512])
                        else:
                            seng = nc.sync if dc == 0 else nc.scalar
                            seng.dma_start(
                                out=o2a[base : base + 128,
                                        jb * D + dc * 512 : jb * D + (dc + 1) * 512],
                                in_=dst)
                if e != EPC - 1:
                    nc.gpsimd.dma_start(
                        out=o2a[e * 128 : (e + 1) * 128, jb * D : (jb + 1) * D], in_=av)
            st[e] = {}

        ident, cms = load_warmup()
        stage_load(0)
        bcs, onesr = load_consts()
        pe_clock_warmup(ident)
        stage_sim(0)
        for e in range(1, EPC):
            stage_load(e)
            stage_finA(e - 1)
            stage_sim(e)
            stage_finB(e - 1)
        stage_finA(EPC - 1)
        stage_finB(EPC - 1)

    nc.compile()
    return nc


def get_nc():
    if "nc" not in _CACHE:
        _CACHE["nc"] = _build_nc()
    return _CACHE["nc"]


def _host_prep(v1, v2, v1_mask, v2_mask):
    """Build per-core input maps (packed per-example layouts) from full inputs."""
    import ml_dtypes

    bf16 = ml_dtypes.bfloat16
    v1 = np.asarray(v1, dtype=np.float32)
    v2 = np.asarray(v2, dtype=np.float32)
    v1_mask = np.asarray(v1_mask).astype(bool)
    v2_mask = np.asarray(v2_mask).astype(bool)

    def pack_t(x):  # [EPC, L, D] -> [EPC*128, ND*L]; row p = concat chunk rows
        return np.ascontiguousarray(
            x.transpose(0, 2, 1).reshape(EPC, ND, 128, L).transpose(0, 2, 1, 3)
            .reshape(EPC * 128, ND * L))

    def pack_n(x):  # [EPC, L, D] -> [EPC*128, NB*D]; row p = concat block rows
        return np.ascontiguousarray(
            x.reshape(EPC, NB, 128, D).transpose(0, 2, 1, 3).reshape(EPC * 128, NB * D))

    in_maps = []
    for k in range(NCORES):
        sl = slice(EPC * k, EPC * (k + 1))
        v1c, v2c = v1[sl], v2[sl]
        m1 = v1_mask[sl]
        m2 = v2_mask[sl]
        b1 = np.where(m1, np.float32(-1e30), np.float32(0.0)).astype(np.float32)
        b2 = np.where(m2, np.float32(-1e30), np.float32(0.0)).astype(np.float32)
        bcol = np.ascontiguousarray(b1.reshape(EPC, NB, 128).transpose(2, 0, 1).reshape(128, EPC * NB))
        b2rep = np.repeat(b2[:, None, :], 128, axis=1).reshape(EPC * 128, L)
        k1 = (~m1).astype(np.float32).reshape(EPC, NB, 128).transpose(2, 0, 1).reshape(128, EPC * NB)
        k2 = (~m2).astype(np.float32).reshape(EPC, NB, 128).transpose(2, 0, 1).reshape(128, EPC * NB)
        in_maps.append(
            {
                "v1t": pack_t(v1c).astype(np.float16),
                "v2t": pack_t(v2c).astype(np.float16),
                "v2n": pack_n(v2c).astype(np.float16),
                "v1n": pack_n(v1c).astype(bf16),
                "b2r": np.ascontiguousarray(b2rep),
                "bcol": bcol,
                "ones2": np.ones((128, 2), bf16),
                "cm": np.ascontiguousarray(np.concatenate([k1, k2], axis=1)),
                "idh": np.eye(128, dtype=np.float16),
            }
        )
    return in_maps


def kernel(v1, v2, v1_mask, v2_mask):
    global LAST_RESULTS
    from concourse.bass_utils import run_bass_kernel_spmd

    nc = get_nc()
    in_maps = _host_prep(v1, v2, v1_mask, v2_mask)
    res = run_bass_kernel_spmd(nc, in_maps, list(range(NCORES)))
    LAST_RESULTS = res

    def unpack(name):
        parts = []
        for k in range(NCORES):
            arr = res.results[k][name].astype(np.float32)
            parts.append(arr.reshape(EPC, 128, NB, D).transpose(0, 2, 1, 3).reshape(EPC, L, D))
        return np.concatenate(parts, axis=0)

    return unpack("o1"), unpack("o2")

